# revision 2
# baseline (speedup 1.0000x reference)
"""Trainium2 Bass kernel for nn_Attention_21792664060632.

GQA attention (32 q heads, 8 kv heads, d=64, s=2048, hidden=2048, causal,
interleaved RoPE with random cos/sin) sharded tensor-parallel over 8
NeuronCores: core c owns q heads {c, c+8, c+16, c+24} (all of which use kv
head c under the reference's jnp.tile GQA mapping) plus kv head c.  Each
core computes a partial output x-projection -> rope -> attention -> @Wo_rows
and the host sums the 8 partials.

Per-core dataflow (all matmuls in float32r, transposes in exact float32):
  A. x row-chunk [128, 2048] -> PE-transpose -> xT tiles (hidden on
     partitions)
  B. QKV projection: psum[128, 384] = sum_k xT[k].T @ Wcat[k]
     (Wcat = [Wq 4 heads | Wk | Wv] columns), RoPE applied in natural
     layout (pairs on free dim), V kept natural with a ones column
  C. PE-transpose rope'd Q,K -> QT pairs [128, 2048] (2 heads stacked),
     KT [128, 2048] (kv head replicated in both partition halves)
  D. per head-pair, per 512-wide q tile: S^T[k,q] matmuls (two heads run
     concurrently in disjoint PE row groups), exp on ScalarE, causal mask,
     AV matmuls with ones-augmented V (M=65) accumulate att output and
     softmax denominators together; normalize via reciprocal + gpsimd
     partition-broadcast
  E. out_partial[s,:] = sum_pair avT_pair.T @ Wo_pair rows
"""

import sys

sys.path.insert(0, "/opt/trn_rl_repo")

import numpy as np

HEADS, KV_HEADS, HEAD_DIM = 32, 8, 64
S, HID = 2048, 2048
NCORES = 8
SC = S // 128  # 16 s-chunks
KC = HID // 128  # 16 hidden-chunks
NQT = S // 512  # 4 q-tiles

_CACHE = {}


def _build_nc():
    import concourse.bacc as bacc
    import concourse.mybir as mybir
    import concourse.tile as tile
    from concourse.masks import make_identity

    F32 = mybir.dt.float32
    F32R = mybir.dt.float32r
    EXP = mybir.ActivationFunctionType.Exp
    MULT = mybir.AluOpType.mult
    SUB = mybir.AluOpType.subtract
    ADD = mybir.AluOpType.add

    nc = bacc.Bacc("TRN2", target_bir_lowering=False, debug=False)

    X = nc.dram_tensor("x", [S, HID], F32, kind="ExternalInput")
    WCAT = nc.dram_tensor("wcat", [HID, 384], F32R, kind="ExternalInput")
    WO = nc.dram_tensor("wo", [256, HID], F32R, kind="ExternalInput")
    COS = nc.dram_tensor("cos", [S, 32], F32, kind="ExternalInput")
    SIN = nc.dram_tensor("sin", [S, 32], F32, kind="ExternalInput")
    OUT = nc.dram_tensor("out", [S, HID], F32, kind="ExternalOutput")

    with tile.TileContext(nc) as tc:
        with (
            tc.tile_pool(name="const", bufs=1) as const,
            tc.tile_pool(name="weights", bufs=1) as wpool,
            tc.tile_pool(name="persist", bufs=1) as persist,
        ):
            ident = const.tile([128, 128], F32)
            make_identity(nc, ident[:])
            mask01 = const.tile([128, 128], F32)
            nc.gpsimd.memset(mask01[:], 1.0)
            # keep only q >= k: free index (q) >= partition index (k)
            nc.gpsimd.affine_select(
                out=mask01[:], in_=mask01[:],
                compare_op=mybir.AluOpType.is_ge,
                fill=0.0, base=0,
                pattern=[[1, 128]], channel_multiplier=-1,
            )
            zeros = const.tile([128, 512], F32)
            nc.vector.memset(zeros[:], 0.0)
            zeros_r = const.tile([128, 512], F32R)
            nc.vector.tensor_copy(zeros_r[:], zeros[:])
            mask01r = const.tile([128, 128], F32R)
            nc.vector.tensor_copy(mask01r[:], mask01[:])
            ones_col = const.tile([128, 1], F32)
            nc.vector.memset(ones_col[:], 1.0)
            ones_r = const.tile([128, 1], F32R)
            nc.vector.tensor_copy(ones_r[:], ones_col[:])
            cos_sb = const.tile([128, SC, 32], F32)
            sin_sb = const.tile([128, SC, 32], F32)
            nc.sync.dma_start(cos_sb[:], COS[:].rearrange("(c p) f -> p c f", p=128))
            nc.sync.dma_start(sin_sb[:], SIN[:].rearrange("(c p) f -> p c f", p=128))

            wcat_sb = wpool.tile([128, KC, 384], F32R)
            nc.sync.dma_start(wcat_sb[:], WCAT[:].rearrange("(c p) n -> p c n", p=128))
            wo_sb = wpool.tile([128, 2, HID], F32R)
            nc.sync.dma_start(wo_sb[:], WO[:].rearrange("(c p) n -> p c n", p=128))

            # persistent transposed activations (f32r for fast matmul)
            qt0 = persist.tile([128, S], F32R)  # heads pair 0 (rows 0:64, 64:128)
            qt1 = persist.tile([128, S], F32R)  # heads pair 1
            kt = persist.tile([128, S], F32R)  # kv head replicated in both halves
            v_sb = persist.tile([128, SC, 65], F32R)  # V natural + ones column
            avt0 = persist.tile([128, S], F32R)  # normalized attn out, pair 0
            avt1 = persist.tile([128, S], F32R)
            qts = [qt0, qt1]
            avts = [avt0, avt1]

            # ---------------- stages A+B+C: projection + rope + transposes
            with (
                tc.tile_pool(name="abc", bufs=2) as abc,
                tc.tile_pool(name="ropet", bufs=2) as ropet,
                tc.tile_pool(name="pst", bufs=3, space="PSUM") as pst,
                tc.tile_pool(name="psq", bufs=2, space="PSUM") as psq,
            ):
                for si in range(SC):
                    x_tile = abc.tile([128, HID], F32, tag="xt")
                    nc.sync.dma_start(x_tile[:], X[si * 128:(si + 1) * 128, :])
                    xt_big = abc.tile([128, KC, 128], F32R, tag="xbig")
                    for kc in range(KC):
                        tp = pst.tile([128, 128], F32, tag="tp")
                        nc.tensor.transpose(
                            tp[:], x_tile[:, kc * 128:(kc + 1) * 128], ident[:]
                        )
                        nc.any.tensor_copy(xt_big[:, kc, :], tp[:])
                    qkv = psq.tile([128, 384], F32, tag="qkv")
                    for kc in range(KC):
                        nc.tensor.matmul(
                            qkv[:], xt_big[:, kc, :], wcat_sb[:, kc, :],
                            start=(kc == 0), stop=(kc == KC - 1),
                        )
                    # ---- RoPE on Q (4 heads) + K (1 head): cols 0:320
                    # pairs interleaved on free dim; group view [128, 5, 32]
                    qk = qkv[:, 0:320].rearrange("p (g i t) -> p g i t", g=5, t=2)
                    q1 = qk[:, :, :, 0]
                    q2 = qk[:, :, :, 1]
                    cs = cos_sb[:, si, None, :].to_broadcast([128, 5, 32])
                    sn = sin_sb[:, si, None, :].to_broadcast([128, 5, 32])
                    t1 = ropet.tile([128, 5, 32], F32, tag="t1")
                    t2 = ropet.tile([128, 5, 32], F32, tag="t2")
                    t3 = ropet.tile([128, 5, 32], F32, tag="t3")
                    t4 = ropet.tile([128, 5, 32], F32, tag="t4")
                    nc.vector.tensor_tensor(t1[:], q1, cs, MULT)
                    nc.vector.tensor_tensor(t2[:], q2, sn, MULT)
                    nc.vector.tensor_tensor(t3[:], q1, sn, MULT)
                    nc.vector.tensor_tensor(t4[:], q2, cs, MULT)
                    rot = ropet.tile([128, 320], F32, tag="rot")
                    rv = rot[:].rearrange("p (g i t) -> p g i t", g=5, t=2)
                    nc.vector.tensor_tensor(rv[:, :, :, 0], t1[:], t2[:], SUB)
                    nc.vector.tensor_tensor(rv[:, :, :, 1], t3[:], t4[:], ADD)
                    # ---- V natural + ones column
                    nc.any.tensor_copy(v_sb[:, si, 0:64], qkv[:, 320:384])
                    nc.any.tensor_copy(v_sb[:, si, 64:65], ones_r[:])
                    # ---- transposes: Q pairs and K
                    for pr in range(2):
                        tq = pst.tile([128, 128], F32, tag="tp")
                        nc.tensor.transpose(
                            tq[:], rot[:, pr * 128:(pr + 1) * 128], ident[:]
                        )
                        nc.any.tensor_copy(
                            qts[pr][:, si * 128:(si + 1) * 128], tq[:]
                        )
                    tk = pst.tile([64, 128], F32, tag="tk")
                    nc.tensor.transpose(tk[:], rot[:, 256:320], ident[:])
                    nc.any.tensor_copy(kt[0:64, si * 128:(si + 1) * 128], tk[:])
                    nc.any.tensor_copy(kt[64:128, si * 128:(si + 1) * 128], tk[:])

            # ---------------- stage D: attention
            with (
                tc.tile_pool(name="pd", bufs=3) as pd,
                tc.tile_pool(name="nrm", bufs=2) as nrm,
                tc.tile_pool(name="pss", bufs=2, space="PSUM") as pss,
                tc.tile_pool(name="psav", bufs=2, space="PSUM") as psav,
            ):
                for pr in range(2):
                    qt = qts[pr]
                    for qj in range(NQT):
                        q0 = qj * 512
                        kimax = 4 * qj + 3
                        av_a = psav.tile([65, 512], F32, tag="ava")
                        av_b = psav.tile([65, 512], F32, tag="avb")
                        avs = (av_a, av_b)
                        for ki in range(kimax + 1):
                            d = ki - 4 * qj  # diag block offset if >= 0
                            qoff = 0 if d < 0 else d * 128
                            for h, (st_tag, p_tag) in enumerate(
                                (("sta", "pa"), ("stb", "pb"))
                            ):
                                hp = h * 64  # partition base for this head
                                st = pss.tile([128, 512], F32, tag=st_tag)
                                nc.tensor.matmul(
                                    st[:, qoff:512],
                                    kt[hp:hp + 64, ki * 128:(ki + 1) * 128],
                                    qt[hp:hp + 64, q0 + qoff:q0 + 512],
                                    start=True, stop=True,
                                )
                                p = pd.tile([128, 512], F32R, tag=p_tag)
                                if d > 0:
                                    nc.any.tensor_copy(
                                        p[:, 0:qoff], zeros_r[:, 0:qoff]
                                    )
                                nc.scalar.activation(
                                    p[:, qoff:512], st[:, qoff:512], EXP,
                                    scale=0.125,
                                )
                                if d >= 0:
                                    nc.vector.tensor_tensor(
                                        p[:, qoff:qoff + 128],
                                        p[:, qoff:qoff + 128],
                                        mask01r[:], MULT,
                                    )
                                nc.tensor.matmul(
                                    avs[h][:],
                                    v_sb[:, ki, :],
                                    p[:],
                                    start=(ki == 0), stop=(ki == kimax),
                                )
                        # normalize: row 64 of av psum is the denominator
                        for h in range(2):
                            hp = h * 64
                            den = nrm.tile([1, 512], F32, tag=f"den{h}")
                            nc.any.tensor_copy(den[:], avs[h][64:65, :])
                            rec = nrm.tile([1, 512], F32, tag=f"rec{h}")
                            nc.vector.reciprocal_approx_fast(rec[:], den[:])
                            bc = nrm.tile([64, 512], F32, tag=f"bc{h}")
                            nc.gpsimd.partition_broadcast(bc[:], rec[0:1, :])
                            nc.vector.tensor_tensor(
                                avts[pr][hp:hp + 64, q0:q0 + 512],
                                avs[h][0:64, :], bc[:], MULT,
                            )

            # ---------------- stage E: output projection (partial sums)
            with (
                tc.tile_pool(name="pe", bufs=3) as pe,
                tc.tile_pool(name="pso", bufs=4, space="PSUM") as pso,
            ):
                for si in range(SC):
                    for nj in range(4):
                        ops = pso.tile([128, 512], F32, tag="o")
                        nc.tensor.matmul(
                            ops[:],
                            avt0[:, si * 128:(si + 1) * 128],
                            wo_sb[:, 0, nj * 512:(nj + 1) * 512],
                            start=True, stop=False,
                        )
                        nc.tensor.matmul(
                            ops[:],
                            avt1[:, si * 128:(si + 1) * 128],
                            wo_sb[:, 1, nj * 512:(nj + 1) * 512],
                            start=False, stop=True,
                        )
                        osb = pe.tile([128, 512], F32, tag="ob")
                        nc.any.tensor_copy(osb[:], ops[:])
                        nc.sync.dma_start(
                            OUT[si * 128:(si + 1) * 128, nj * 512:(nj + 1) * 512],
                            osb[:],
                        )

    nc.compile()
    return nc


def _shard_inputs(x, cos, sin, Wq, Wk, Wv, Wo):
    """Build the 8 per-core input maps (tensor-parallel by head groups)."""
    in_maps = []
    for c in range(NCORES):
        heads = [c, c + 8, c + 16, c + 24]
        wq_cols = np.concatenate(
            [Wq[:, h * 64:(h + 1) * 64] for h in heads], axis=1
        )
        wcat = np.concatenate(
            [wq_cols, Wk[:, c * 64:(c + 1) * 64], Wv[:, c * 64:(c + 1) * 64]],
            axis=1,
        ).astype(np.float32)
        wo_rows = np.concatenate(
            [Wo[h * 64:(h + 1) * 64, :] for h in heads], axis=0
        ).astype(np.float32)
        in_maps.append(
            {
                "x": np.ascontiguousarray(x),
                "wcat": np.ascontiguousarray(wcat),
                "wo": np.ascontiguousarray(wo_rows),
                "cos": np.ascontiguousarray(cos),
                "sin": np.ascontiguousarray(sin),
            }
        )
    return in_maps


def run(inputs, trace=False):
    """Run on all 8 cores; returns (full_output [1,S,HID], BassKernelResults)."""
    from concourse.bass_utils import run_bass_kernel_spmd

    x = np.asarray(inputs["x"], dtype=np.float32)[0]
    cos = np.asarray(inputs["cos"], dtype=np.float32)
    sin = np.asarray(inputs["sin"], dtype=np.float32)
    Wq = np.asarray(inputs["Wq"], dtype=np.float32)
    Wk = np.asarray(inputs["Wk"], dtype=np.float32)
    Wv = np.asarray(inputs["Wv"], dtype=np.float32)
    Wo = np.asarray(inputs["Wo"], dtype=np.float32)

    if "nc" not in _CACHE:
        _CACHE["nc"] = _build_nc()
    nc = _CACHE["nc"]

    in_maps = _shard_inputs(x, cos, sin, Wq, Wk, Wv, Wo)
    res = run_bass_kernel_spmd(
        nc, in_maps, core_ids=list(range(NCORES)), trace=trace
    )
    out = np.zeros((S, HID), dtype=np.float32)
    for r in res.results:
        out += r["out"]
    return out.reshape(1, S, HID), res


def kernel(**inputs) -> np.ndarray:
    out, _ = run(inputs, trace=False)
    return out


# revision 11
# speedup vs baseline: 1.6229x; 1.6229x over previous
"""Trainium2 Bass kernel for nn_Attention_21792664060632.

GQA attention (32 q heads, 8 kv heads, d=64, s=2048, hidden=2048, causal,
interleaved RoPE with random cos/sin) sharded tensor-parallel over 8
NeuronCores: core c owns q heads {c, c+8, c+16, c+24} (all of which use kv
head c under the reference's jnp.tile GQA mapping) plus kv head c.  Each
core computes a partial output x-projection -> rope -> attention -> @Wo_rows
and the host sums the 8 partials.

Per-core dataflow (all matmuls in float32r, transposes in exact float32):
  A. x row-chunk [128, 2048] -> PE-transpose -> xT tiles (hidden on
     partitions)
  B. QKV projection: psum[128, 384] = sum_k xT[k].T @ Wcat[k]
     (Wcat = [Wq 4 heads | Wk | Wv] columns), RoPE applied in natural
     layout (pairs on free dim), V kept natural with a ones column
  C. PE-transpose rope'd Q,K -> QT pairs [128, 2048] (2 heads stacked),
     KT [128, 2048] (kv head replicated in both partition halves)
  D. per head-pair, per 512-wide q tile: S^T[k,q] matmuls (two heads run
     concurrently in disjoint PE row groups), exp on ScalarE, causal mask,
     AV matmuls with ones-augmented V (M=65) accumulate att output and
     softmax denominators together; normalize via reciprocal + gpsimd
     partition-broadcast
  E. out_partial[s,:] = sum_pair avT_pair.T @ Wo_pair rows
"""

import sys

sys.path.insert(0, "/opt/trn_rl_repo")

import numpy as np

HEADS, KV_HEADS, HEAD_DIM = 32, 8, 64
S, HID = 2048, 2048
NCORES = 8
SC = S // 128  # 16 s-chunks
KC = HID // 128  # 16 hidden-chunks
NQT = S // 512  # 4 q-tiles

_CACHE = {}


def _build_nc():
    import concourse.bacc as bacc
    import concourse.mybir as mybir
    import concourse.tile as tile
    from concourse.masks import make_identity

    F32 = mybir.dt.float32
    F32R = mybir.dt.float32r
    EXP = mybir.ActivationFunctionType.Exp
    MULT = mybir.AluOpType.mult
    SUB = mybir.AluOpType.subtract
    ADD = mybir.AluOpType.add

    nc = bacc.Bacc("TRN2", target_bir_lowering=False, debug=False)

    XT = nc.dram_tensor("xt", [HID, S], F32R, kind="ExternalInput")
    WCAT = nc.dram_tensor("wcat", [HID, 384], F32R, kind="ExternalInput")
    WO = nc.dram_tensor("wo", [256, HID], F32R, kind="ExternalInput")
    COS = nc.dram_tensor("cos", [S, 32], F32, kind="ExternalInput")
    SIN = nc.dram_tensor("sin", [S, 32], F32, kind="ExternalInput")
    OUT = nc.dram_tensor("out", [S, HID], F32, kind="ExternalOutput")

    with tile.TileContext(nc) as tc:
        with (
            tc.tile_pool(name="const", bufs=1) as const,
            tc.tile_pool(name="weights", bufs=1) as wpool,
            tc.tile_pool(name="persist", bufs=1) as persist,
        ):
            ident = const.tile([128, 128], F32)
            make_identity(nc, ident[:])
            mask01 = const.tile([128, 128], F32)
            nc.gpsimd.memset(mask01[:], 1.0)
            # keep only q >= k: free index (q) >= partition index (k)
            nc.gpsimd.affine_select(
                out=mask01[:], in_=mask01[:],
                compare_op=mybir.AluOpType.is_ge,
                fill=0.0, base=0,
                pattern=[[1, 128]], channel_multiplier=-1,
            )
            mask01r = const.tile([128, 128], F32R)
            nc.vector.tensor_copy(mask01r[:], mask01[:])
            ones_col = const.tile([128, 1], F32)
            nc.vector.memset(ones_col[:], 1.0)
            ones_r = const.tile([128, 1], F32R)
            nc.vector.tensor_copy(ones_r[:], ones_col[:])
            cos_sb = const.tile([128, SC, 32], F32)
            sin_sb = const.tile([128, SC, 32], F32)
            nc.sync.dma_start(cos_sb[:], COS[:].rearrange("(c p) f -> p c f", p=128))
            nc.sync.dma_start(sin_sb[:], SIN[:].rearrange("(c p) f -> p c f", p=128))

            wcat_sb = wpool.tile([128, KC, 384], F32R)
            nc.sync.dma_start(wcat_sb[:], WCAT[:].rearrange("(c p) n -> p c n", p=128))
            wo_sb = wpool.tile([128, 2, HID], F32R)
            nc.sync.dma_start(wo_sb[:], WO[:].rearrange("(c p) n -> p c n", p=128))

            # persistent transposed activations (f32r for fast matmul)
            qt0 = persist.tile([128, S], F32R)  # heads pair 0 (rows 0:64, 64:128)
            qt1 = persist.tile([128, S], F32R)  # heads pair 1
            kt = persist.tile([128, S], F32R)  # kv head replicated in both halves
            v_sb = persist.tile([128, SC, 65], F32R)  # V natural + ones column
            avt0 = persist.tile([128, S], F32R)  # normalized attn out, pair 0
            avt1 = persist.tile([128, S], F32R)
            qts = [qt0, qt1]
            avts = [avt0, avt1]

            # ---------------- stages A+B+C: projection + rope + transposes
            with (
                tc.tile_pool(name="abc", bufs=2) as abc,
                tc.tile_pool(name="ropet", bufs=2) as ropet,
                tc.tile_pool(name="pst", bufs=3, space="PSUM") as pst,
                tc.tile_pool(name="psq", bufs=2, space="PSUM") as psq,
            ):
                for si in range(SC):
                    x_tile = abc.tile([128, HID], F32, tag="xt")
                    nc.sync.dma_start(x_tile[:], X[si * 128:(si + 1) * 128, :])
                    xt_big = abc.tile([128, KC, 128], F32R, tag="xbig", bufs=3)
                    for kc in range(KC):
                        tp = pst.tile([128, 128], F32, tag="tp")
                        nc.tensor.transpose(
                            tp[:], x_tile[:, kc * 128:(kc + 1) * 128], ident[:]
                        )
                        nc.any.tensor_copy(xt_big[:, kc, :], tp[:])
                    qkv = psq.tile([128, 384], F32, tag="qkv")
                    for kc in range(KC):
                        nc.tensor.matmul(
                            qkv[:], xt_big[:, kc, :], wcat_sb[:, kc, :],
                            start=(kc == 0), stop=(kc == KC - 1),
                        )
                    # ---- RoPE on Q (4 heads) + K (1 head): cols 0:320
                    # pairs interleaved on free dim; group view [128, 5, 32]
                    qk = qkv[:, 0:320].rearrange("p (g i t) -> p g i t", g=5, t=2)
                    q1 = qk[:, :, :, 0]
                    q2 = qk[:, :, :, 1]
                    cs = cos_sb[:, si, None, :].to_broadcast([128, 5, 32])
                    sn = sin_sb[:, si, None, :].to_broadcast([128, 5, 32])
                    t1 = ropet.tile([128, 5, 32], F32, tag="t1")
                    t2 = ropet.tile([128, 5, 32], F32, tag="t2")
                    t3 = ropet.tile([128, 5, 32], F32, tag="t3")
                    t4 = ropet.tile([128, 5, 32], F32, tag="t4")
                    nc.vector.tensor_tensor(t1[:], q1, cs, MULT)
                    nc.vector.tensor_tensor(t2[:], q2, sn, MULT)
                    nc.vector.tensor_tensor(t3[:], q1, sn, MULT)
                    nc.vector.tensor_tensor(t4[:], q2, cs, MULT)
                    rot = ropet.tile([128, 320], F32, tag="rot")
                    rv = rot[:].rearrange("p (g i t) -> p g i t", g=5, t=2)
                    nc.vector.tensor_tensor(rv[:, :, :, 0], t1[:], t2[:], SUB)
                    nc.vector.tensor_tensor(rv[:, :, :, 1], t3[:], t4[:], ADD)
                    # ---- V natural + ones column
                    nc.any.tensor_copy(v_sb[:, si, 0:64], qkv[:, 320:384])
                    nc.vector.tensor_copy(v_sb[:, si, 64:65], ones_r[:])
                    # ---- transposes: Q pairs and K
                    for pr in range(2):
                        tq = pst.tile([128, 128], F32, tag="tp")
                        nc.tensor.transpose(
                            tq[:], rot[:, pr * 128:(pr + 1) * 128], ident[:]
                        )
                        nc.any.tensor_copy(
                            qts[pr][:, si * 128:(si + 1) * 128], tq[:]
                        )
                    tk = pst.tile([64, 128], F32, tag="tk")
                    nc.tensor.transpose(tk[:], rot[:, 256:320], ident[:])
                    nc.any.tensor_copy(kt[0:64, si * 128:(si + 1) * 128], tk[:])
                    nc.any.tensor_copy(kt[64:128, si * 128:(si + 1) * 128], tk[:])

            # ---------------- stage D: attention
            with (
                tc.tile_pool(name="pd", bufs=3) as pd,
                tc.tile_pool(name="nrm", bufs=1) as nrm,
                tc.tile_pool(name="pss", bufs=2, space="PSUM") as pss,
                tc.tile_pool(name="psav", bufs=2, space="PSUM") as psav,
            ):
                for pr in range(2):
                    qt = qts[pr]
                    for qj in range(NQT):
                        q0 = qj * 512
                        kimax = 4 * qj + 3
                        av_a = psav.tile([65, 512], F32, tag="ava")
                        av_b = psav.tile([65, 512], F32, tag="avb")
                        avs = (av_a, av_b)
                        for ki in range(kimax + 1):
                            d = ki - 4 * qj  # diag block offset if >= 0
                            qoff = 0 if d < 0 else d * 128
                            for h, (st_tag, p_tag) in enumerate(
                                (("sta", "pa"), ("stb", "pb"))
                            ):
                                hp = h * 64  # partition base for this head
                                st = pss.tile([128, 512], F32, tag=st_tag)
                                nc.tensor.matmul(
                                    st[:, qoff:512],
                                    kt[hp:hp + 64, ki * 128:(ki + 1) * 128],
                                    qt[hp:hp + 64, q0 + qoff:q0 + 512],
                                    start=True, stop=True,
                                )
                                p = pd.tile([128, 512], F32R, tag=p_tag)
                                if d > 0:
                                    nc.any.tensor_copy(
                                        p[:, 0:qoff], zeros_r[:, 0:qoff]
                                    )
                                nc.scalar.activation(
                                    p[:, qoff:512], st[:, qoff:512], EXP,
                                    scale=0.125,
                                )
                                if d >= 0:
                                    nc.vector.tensor_tensor(
                                        p[:, qoff:qoff + 128],
                                        p[:, qoff:qoff + 128],
                                        mask01r[:], MULT,
                                    )
                                nc.tensor.matmul(
                                    avs[h][:],
                                    v_sb[:, ki, :],
                                    p[:],
                                    start=(ki == 0), stop=(ki == kimax),
                                )
                        # normalize: row 64 of av psum is the denominator
                        for h in range(2):
                            hp = h * 64
                            den = nrm.tile([1, 512], F32, tag=f"den{h}")
                            nc.vector.tensor_copy(den[:], avs[h][64:65, :])
                            rec = nrm.tile([1, 512], F32, tag=f"rec{h}")
                            nc.vector.reciprocal_approx_fast(rec[:], den[:])
                            bc = nrm.tile([64, 512], F32, tag=f"bc{h}")
                            nc.gpsimd.partition_broadcast(bc[:], rec[0:1, :])
                            nc.vector.tensor_tensor(
                                avts[pr][hp:hp + 64, q0:q0 + 512],
                                avs[h][0:64, :], bc[:], MULT,
                            )

            # ---------------- stage E: output projection (partial sums)
            with (
                tc.tile_pool(name="pe", bufs=3) as pe,
                tc.tile_pool(name="pso", bufs=4, space="PSUM") as pso,
            ):
                for si in range(SC):
                    for nj in range(4):
                        ops = pso.tile([128, 512], F32, tag="o")
                        nc.tensor.matmul(
                            ops[:],
                            avt0[:, si * 128:(si + 1) * 128],
                            wo_sb[:, 0, nj * 512:(nj + 1) * 512],
                            start=True, stop=False,
                        )
                        nc.tensor.matmul(
                            ops[:],
                            avt1[:, si * 128:(si + 1) * 128],
                            wo_sb[:, 1, nj * 512:(nj + 1) * 512],
                            start=False, stop=True,
                        )
                        osb = pe.tile([128, 512], F32, tag="ob")
                        nc.any.tensor_copy(osb[:], ops[:])
                        nc.sync.dma_start(
                            OUT[si * 128:(si + 1) * 128, nj * 512:(nj + 1) * 512],
                            osb[:],
                        )

    nc.compile()
    return nc


def _shard_inputs(x, cos, sin, Wq, Wk, Wv, Wo):
    """Build the 8 per-core input maps (tensor-parallel by head groups)."""
    xt = np.ascontiguousarray(x.T)
    in_maps = []
    for c in range(NCORES):
        heads = [c, c + 8, c + 16, c + 24]
        wq_cols = np.concatenate(
            [Wq[:, h * 64:(h + 1) * 64] for h in heads], axis=1
        )
        wcat = np.concatenate(
            [wq_cols, Wk[:, c * 64:(c + 1) * 64], Wv[:, c * 64:(c + 1) * 64]],
            axis=1,
        ).astype(np.float32)
        wo_rows = np.concatenate(
            [Wo[h * 64:(h + 1) * 64, :] for h in heads], axis=0
        ).astype(np.float32)
        in_maps.append(
            {
                "xt": xt,
                "wcat": np.ascontiguousarray(wcat),
                "wo": np.ascontiguousarray(wo_rows),
                "cos": np.ascontiguousarray(cos),
                "sin": np.ascontiguousarray(sin),
            }
        )
    return in_maps


def run(inputs, trace=False):
    """Run on all 8 cores; returns (full_output [1,S,HID], BassKernelResults)."""
    from concourse.bass_utils import run_bass_kernel_spmd

    x = np.asarray(inputs["x"], dtype=np.float32)[0]
    cos = np.asarray(inputs["cos"], dtype=np.float32)
    sin = np.asarray(inputs["sin"], dtype=np.float32)
    Wq = np.asarray(inputs["Wq"], dtype=np.float32)
    Wk = np.asarray(inputs["Wk"], dtype=np.float32)
    Wv = np.asarray(inputs["Wv"], dtype=np.float32)
    Wo = np.asarray(inputs["Wo"], dtype=np.float32)

    if "nc" not in _CACHE:
        _CACHE["nc"] = _build_nc()
    nc = _CACHE["nc"]

    in_maps = _shard_inputs(x, cos, sin, Wq, Wk, Wv, Wo)
    res = run_bass_kernel_spmd(
        nc, in_maps, core_ids=list(range(NCORES)), trace=trace
    )
    out = np.zeros((S, HID), dtype=np.float32)
    for r in res.results:
        out += r["out"]
    return out.reshape(1, S, HID), res


def kernel(**inputs) -> np.ndarray:
    out, _ = run(inputs, trace=False)
    return out
